# revision 24
# baseline (speedup 1.0000x reference)
"""Block-local sparse attention (LSG-style) on 8 TRN2 NeuronCores.

Sharding: the 32 (n, h) pairs are split 4-per-core (data/head parallel, no
collectives). Host-side numpy prep re-lays-out the inputs so the device
kernel needs no transposes, all bf16:

  - qt : Q^T per head [64, T]
  - lkt/skt/gkt: local/sparse/global K^T, token-padded with zeros
  - lv/sv/gv: V with a ones column appended (col 64), chunked [128, c, 65],
    and every row scaled by exp(mask): softmax(QK/8 + m) @ V is computed as
    sum_t exp(s_t) e^{m_t} [V_t, 1], then a divide by the accumulated last
    column — exact for any additive mask, and pad tokens (e^{m}=0) vanish
    from both numerator and denominator, so no mask row and no
    max-subtraction are needed (|QK|/8 is O(5), well within fp32 exp range).
    sv additionally holds 4 phase-shifted copies so the 32-token-granular
    sparse windows always start at partition 0.

The device processes query-block PAIRS: 9 score matmuls per pair into a
3-bank PSUM region [128, 1536] laid out so no matmul output crosses a
bank, exp(S/8) on ACT split in two ops (cols 0:1024 after 6 matmuls,
1024:1536 after all 9) so the PV matmuls can start on the sparse/global/
loc1 chunks while the local exp still runs, then 12 PV matmuls (N=65)
into [q, V|Z] and a per-row PSUM divide on DVE (no reciprocal RAW, no
drain).

Raw bass with hand-placed semaphores (walrus: at most one sem wait per
matmul/ACT instruction). Queue assignment keeps the stores from ever
queueing behind the multi-MB slot loads: input loads issue from the idle
GpSimd queue (K-side and V-side tensors on separate counting semaphores
so scores start after 1.26MB instead of 2.57MB), output stores (one
merged DMA per pair) from the Sync queue. Scores of pair p+2 issue right
after PV of pair p; optional filler matmuls (FILL x 512 cols into the
just-consumed psS bank, harmlessly overwritten by the next scores'
start=True) keep the PE gap-free so the DVFS p-state can ramp
1.2 GHz -> 2.4 GHz.
"""

from contextlib import ExitStack

import numpy as np

import concourse.bass as bass
import concourse.mybir as mybir
from concourse.bass_utils import run_bass_kernel_spmd

N, H, T, D = 2, 16, 4096, 64
B = 128          # query block
NB = T // B      # 32
G = 64           # global tokens
TSP = T // 4     # sparse tokens (1024)
NH = N * H       # 32
NCORES = 8
SL = NH // NCORES  # 4 heads per core
NP = SL * NB // 2  # 64 block-pairs per core
PPS = NB // 2      # 16 pairs per slot

LKT_W = T + 2 * B            # 4352 padded local tokens
SKT_W = TSP + 320            # 1344 padded sparse tokens
LV_C = LKT_W // 128          # 34 local V chunks
SV_C = 11                    # sparse V chunks per phase

F32 = mybir.dt.float32
BF16 = mybir.dt.bfloat16
GE = "sem-ge"

FILL = 0  # PE filler matmuls per pair (measured: no DVFS ramp on this HW)

# column layout of the per-pair score/prob tile [128, 1536] (3 PSUM banks;
# regions never cross a 512-col bank boundary).  exp op A covers [0:1024),
# exp op B covers [1024:1536).
C_SP1A, C_SP1B = 0, 128
C_SP2A, C_SP2B = 256, 384
C_G = 512        # 256 wide: q of both blocks
C_LOC1 = 768     # 256 wide: local chunk b+1, both blocks
C_LOC0 = 1024    # 128: local chunk b, block A only
C_LOC2 = 1152    # 256 wide: local chunk b+2, both blocks
C_LOC3 = 1408    # 128: local chunk b+3, block B only


def _build_bass():
    nc = bass.Bass("TRN2", num_devices=NCORES, debug=False)

    qt = nc.dram_tensor("qt", [SL, 64, T], BF16, kind="ExternalInput")
    lkt = nc.dram_tensor("lkt", [SL, 64, LKT_W], BF16, kind="ExternalInput")
    skt = nc.dram_tensor("skt", [SL, 64, SKT_W], BF16, kind="ExternalInput")
    gkt = nc.dram_tensor("gkt", [SL, 64, 128], BF16, kind="ExternalInput")
    lv = nc.dram_tensor("lv", [SL, 128, LV_C * 65], BF16, kind="ExternalInput")
    sv = nc.dram_tensor("sv", [SL, 128, 4 * SV_C * 65], BF16, kind="ExternalInput")
    gv = nc.dram_tensor("gv", [SL, 128, 65], BF16, kind="ExternalInput")
    o = nc.dram_tensor("o", [SL, T, D], F32, kind="ExternalOutput")

    EXP = mybir.ActivationFunctionType.Exp

    with ExitStack() as es:
        ec = es.enter_context
        # double-buffered inputs (slot parity).  K-side tensors are 128
        # partitions with rows 64:128 zeroed once at startup: keeping every
        # matmul at 128-partition contraction avoids the PE row-mode switch
        # (64p<->128p) that drains the array ~200ns at each phase boundary.
        qt_t = [ec(nc.sbuf_tensor(f"qt_t{i}", [128, T], BF16)) for i in range(2)]
        lkt_t = [ec(nc.sbuf_tensor(f"lkt_t{i}", [128, LKT_W], BF16)) for i in range(2)]
        skt_t = [ec(nc.sbuf_tensor(f"skt_t{i}", [128, SKT_W], BF16)) for i in range(2)]
        gkt_t = [ec(nc.sbuf_tensor(f"gkt_t{i}", [128, 128], BF16)) for i in range(2)]
        lv_t = [ec(nc.sbuf_tensor(f"lv_t{i}", [128, LV_C * 65], BF16)) for i in range(2)]
        sv_t = [ec(nc.sbuf_tensor(f"sv_t{i}", [128, 4 * SV_C * 65], BF16)) for i in range(2)]
        gv_t = [ec(nc.sbuf_tensor(f"gv_t{i}", [128, 65], BF16)) for i in range(2)]
        # double-buffered per-pair working set (pair parity)
        psS = [ec(nc.psum_tensor(f"psS{i}", [128, 1536], F32)) for i in range(2)]  # 3 banks
        pv = [ec(nc.psum_tensor(f"pv{i}", [128, 512], F32)) for i in range(2)]     # 1 bank
        pp = [ec(nc.sbuf_tensor(f"pp{i}", [128, 1536], BF16)) for i in range(2)]
        rec = [ec(nc.sbuf_tensor(f"rec{i}", [128, 2], F32)) for i in range(2)]
        # 8-deep output ring: slot-load DMA bursts delay store packets by up
        # to ~10us in the shared engines; 8 pairs of slack rides that out.
        OBN = 8
        ob = [ec(nc.sbuf_tensor(f"ob{i}", [128, 128], F32)) for i in range(OBN)]

        diK = [ec(nc.semaphore(f"diK{i}")) for i in range(2)]  # K-side loads, slot parity
        diV = [ec(nc.semaphore(f"diV{i}")) for i in range(2)]  # V-side loads, slot parity
        st = ec(nc.semaphore("st"))      # out stores (+16 per store, FIFO)
        iz = ec(nc.semaphore("iz"))      # K-tensor rows 64:128 zeroed
        pe_s = ec(nc.semaphore("pe_s"))  # +2 per pair: score matmuls (6 and 9) done
        pe_v = ec(nc.semaphore("pe_v"))  # +1 per pair: PV matmuls done
        act = ec(nc.semaphore("act"))    # +2 per pair: exp halves done
        dve = ec(nc.semaphore("dve"))    # +1 per pair: divide done
        block = ec(nc.Block(no_gpsimd_drain=True))

        @block.gpsimd
        def _(gpsimd):
            def load_slot(s, gate):
                u = s % 2
                kside = (
                    (skt_t[u], skt[s]),
                    (gkt_t[u], gkt[s]),
                    (qt_t[u], qt[s]),
                    (lkt_t[u], lkt[s]),
                )
                vside = (
                    (gv_t[u], gv[s]),
                    (sv_t[u], sv[s]),
                    (lv_t[u], lv[s]),
                )
                first = True
                for dst, src in kside:
                    d = gpsimd.dma_start(dst[0:64, :], src)
                    if first and gate is not None:
                        d.wait_op(pe_v, gate, GE)
                        first = False
                    d.then_inc(diK[u], 16)
                for dst, src in vside:
                    gpsimd.dma_start(dst[:], src).then_inc(diV[u], 16)

            load_slot(0, None)
            load_slot(1, None)
            load_slot(2, 16)
            load_slot(3, 32)

        def emit_scores(p):
            s, hb = divmod(p, PPS)
            b = 2 * hb
            su = s % 2
            if hb == 0:
                nc.tensor.wait_ge(diK[su], 64 * (s // 2 + 1))
            qA = qt_t[su][:, b * B : (b + 1) * B]
            qB = qt_t[su][:, (b + 1) * B : (b + 2) * B]
            qAB = qt_t[su][:, b * B : (b + 2) * B]
            w1a, w2a = 32 * b, 32 * b + 224
            w1b, w2b = w1a + 32, w2a + 32
            u = p % 2
            mms = (
                (C_SP1A, 128, skt_t[su][:, w1a : w1a + 128], qA),
                (C_SP1B, 128, skt_t[su][:, w1b : w1b + 128], qB),
                (C_SP2A, 128, skt_t[su][:, w2a : w2a + 128], qA),
                (C_SP2B, 128, skt_t[su][:, w2b : w2b + 128], qB),
                (C_G, 256, gkt_t[su][:, :], qAB),
                (C_LOC1, 256, lkt_t[su][:, (b + 1) * B : (b + 2) * B], qAB),
                (C_LOC0, 128, lkt_t[su][:, b * B : (b + 1) * B], qA),
                (C_LOC2, 256, lkt_t[su][:, (b + 2) * B : (b + 3) * B], qAB),
                (C_LOC3, 128, lkt_t[su][:, (b + 3) * B : (b + 4) * B], qB),
            )
            for kk, (col, w, lhsT, rhs) in enumerate(mms):
                mm = nc.tensor.matmul(
                    psS[u][:, col : col + w],
                    lhsT, rhs,
                    start=True, stop=True,
                )
                if kk == 5 or kk == 8:
                    mm.then_inc(pe_s, 1)

        def emit_pv(p):
            s, hb = divmod(p, PPS)
            b = 2 * hb
            u = p % 2
            su = s % 2
            if p >= 2:
                nc.tensor.wait_ge(dve, p - 1)  # pv[u] free
            if hb == 0:
                nc.tensor.wait_ge(diV[su], 48 * (s // 2 + 1))
            sp = []
            for blk in range(2):
                bb = b + blk
                w1, w2 = 32 * bb, 32 * bb + 224
                c1, r1 = divmod(w1, 128)
                c2, r2 = divmod(w2, 128)
                sp.append((((r1 // 32) * SV_C + c1) * 65, ((r2 // 32) * SV_C + c2) * 65))
            outA = pv[u][:, 0:65]
            outB = pv[u][:, 128:193]
            # Sequential accumulation groups (A fully, then B): a start=True
            # marks the surrounding 2KB PSUM zero-region pending-zero, so two
            # interleaved in-flight groups in one bank corrupt each other.
            # (out, pp col, rhs, start, stop)
            mms = (
                (outA, C_SP1A, sv_t[su][:, sp[0][0] : sp[0][0] + 65], True, False),
                (outA, C_SP2A, sv_t[su][:, sp[0][1] : sp[0][1] + 65], False, False),
                (outA, C_G, gv_t[su][:], False, False),
                (outA, C_LOC1, lv_t[su][:, (b + 1) * 65 : (b + 2) * 65], False, False),
                (outA, C_LOC0, lv_t[su][:, b * 65 : b * 65 + 65], False, False),
                (outA, C_LOC2, lv_t[su][:, (b + 2) * 65 : (b + 3) * 65], False, True),
                (outB, C_SP1B, sv_t[su][:, sp[1][0] : sp[1][0] + 65], True, False),
                (outB, C_SP2B, sv_t[su][:, sp[1][1] : sp[1][1] + 65], False, False),
                (outB, C_G + 128, gv_t[su][:], False, False),
                (outB, C_LOC1 + 128, lv_t[su][:, (b + 1) * 65 : (b + 2) * 65], False, False),
                (outB, C_LOC2 + 128, lv_t[su][:, (b + 2) * 65 : (b + 3) * 65], False, False),
                (outB, C_LOC3, lv_t[su][:, (b + 3) * 65 : (b + 4) * 65], False, True),
            )
            for kk, (out, col, rhs, st_, sp_) in enumerate(mms):
                if kk == 0:
                    nc.tensor.wait_ge(act, p + 1)  # exp(p) done
                mm = nc.tensor.matmul(
                    out, pp[u][:, col : col + 128], rhs,
                    start=st_, stop=sp_, skip_group_check=True,
                )
                if kk == 11:
                    mm.then_inc(pe_v, 1)

        def emit_fill(p):
            u = p % 2
            su = (p // PPS) % 2
            for _ in range(FILL):
                nc.tensor.matmul(
                    psS[u][:, 0:512],
                    gkt_t[su][:, :],
                    qt_t[su][:, 0:512],
                    start=True, stop=True, skip_group_check=True,
                )

        @block.tensor
        def _(tensor):
            tensor.wait_ge(iz, 1)
            tensor.wait_ge(diK[0], 64)
            emit_scores(0)
            emit_scores(1)
            for p in range(NP):
                emit_pv(p)
                if FILL and p >= 2:
                    emit_fill(p)
                if p + 2 < NP:
                    emit_scores(p + 2)

        @block.scalar
        def _(scalar):
            # single exp per pair: ACT is the pacing engine once the PE runs
            # at the full 128-partition rate, so one op (one PSUM-access
            # bubble) beats the split.
            for p in range(NP):
                u = p % 2
                if p >= 2:
                    scalar.wait_ge(pe_v, p - 1)  # pp[u] free: PV of p-2 done
                nc.scalar.activation(
                    pp[u][:, 0:1536], psS[u][:, 0:1536], EXP, scale=0.125
                ).wait_op(pe_s, 2 * p + 2, GE).then_inc(act, 1)

        @block.vector
        def _(vector):
            # zero rows 64:128 of the K-side tensors once, overlapped with
            # the slot-0 DMA (DVE is idle until the first normalize anyway)
            for tt in (qt_t[0], qt_t[1], lkt_t[0], lkt_t[1],
                       skt_t[0], skt_t[1], gkt_t[0], gkt_t[1]):
                nc.vector.memzero(tt[64:128, :])
            nc.vector.drain()
            nc.vector.nop().then_inc(iz, 1)
            for p in range(NP):
                u = p % 2
                w = p % OBN
                if p >= OBN:
                    vector.wait_ge(st, 16 * (p - OBN + 1))  # ob[w] stored
                nc.vector.reciprocal(rec[u][:, 0:1], pv[u][:, 64:65]).wait_op(
                    pe_v, p + 1, GE
                )
                nc.vector.reciprocal(rec[u][:, 1:2], pv[u][:, 192:193])
                nc.vector.drain()  # DVE pipeline RAW: rec written, read next
                nc.vector.tensor_mul(
                    ob[w][:, 0:64], pv[u][:, 0:64],
                    rec[u][:, 0:1].broadcast_to([128, 64]),
                )
                nc.vector.tensor_mul(
                    ob[w][:, 64:128], pv[u][:, 128:192],
                    rec[u][:, 1:2].broadcast_to([128, 64]),
                ).then_inc(dve, 1)

        @block.sync
        def _(sync):
            for p in range(NP):
                s, hb = divmod(p, PPS)
                b = 2 * hb
                dst = o[s, b * B : (b + 2) * B, :].rearrange(
                    "(blk q) d -> q blk d", blk=2
                )
                src = ob[p % OBN][:, 0:128].rearrange("q (blk d) -> q blk d", blk=2)
                sync.dma_start(dst, src).wait_op(dve, p + 1, GE).then_inc(st, 16)
            sync.wait_ge(st, 16 * NP)

    return nc


def _prepare(inputs):
    import ml_dtypes

    bf = ml_dtypes.bfloat16
    f = np.float32
    q = np.asarray(inputs["query_layer"], f).reshape(NH, T, D)
    k = np.asarray(inputs["key_layer"], f).reshape(NH, T, D)
    v = np.asarray(inputs["value_layer"], f).reshape(NH, T, D)
    sk = np.asarray(inputs["sparse_key"], f).reshape(NH, TSP, D)
    svv = np.asarray(inputs["sparse_value"], f).reshape(NH, TSP, D)
    gk = np.asarray(inputs["global_key"], f).reshape(NH, G, D)
    gvv = np.asarray(inputs["global_value"], f).reshape(NH, G, D)
    am = np.repeat(np.asarray(inputs["attention_mask"], f)[:, 0, 0, :], H, 0)
    sm = np.repeat(np.asarray(inputs["sparse_mask"], f)[:, 0, 0, :], H, 0)
    gm = np.repeat(np.asarray(inputs["global_mask"], f)[:, 0, 0, :], H, 0)

    qt = np.ascontiguousarray(q.transpose(0, 2, 1)).astype(bf)

    lkt = np.zeros((NH, 64, LKT_W), f)
    lkt[:, :, B : B + T] = k.transpose(0, 2, 1)
    lkt = lkt.astype(bf)

    skt = np.zeros((NH, 64, SKT_W), f)
    skt[:, :, 160 : 160 + TSP] = sk.transpose(0, 2, 1)
    skt = skt.astype(bf)

    gkt = np.zeros((NH, 64, 128), f)
    gkt[:, :, :G] = gk.transpose(0, 2, 1)
    gkt = gkt.astype(bf)

    # V_aug rows scaled by exp(mask); pad rows are all-zero
    em_l = np.zeros((NH, LKT_W), f)
    em_l[:, B : B + T] = np.exp(am)
    lvp = np.zeros((NH, LKT_W, 65), f)
    lvp[:, B : B + T, :64] = v
    lvp[:, :, 64] = 1.0
    lvp *= em_l[:, :, None]
    lvp = np.ascontiguousarray(
        lvp.reshape(NH, LV_C, 128, 65).transpose(0, 2, 1, 3)
    ).reshape(NH, 128, LV_C * 65).astype(bf)

    SVP_W = 96 + SV_C * 128
    em_s = np.zeros((NH, SVP_W), f)
    em_s[:, 160 : 160 + TSP] = np.exp(sm)
    sv_pad = np.zeros((NH, SVP_W, 65), f)
    sv_pad[:, 160 : 160 + TSP, :64] = svv
    sv_pad[:, :, 64] = 1.0
    sv_pad *= em_s[:, :, None]
    svp = np.empty((NH, 4, 128, SV_C, 65), f)
    for p in range(4):
        svp[:, p] = (
            sv_pad[:, 32 * p : 32 * p + SV_C * 128]
            .reshape(NH, SV_C, 128, 65)
            .transpose(0, 2, 1, 3)
        )
    svp = np.ascontiguousarray(svp.transpose(0, 2, 1, 3, 4)).reshape(
        NH, 128, 4 * SV_C * 65
    ).astype(bf)

    gvp = np.zeros((NH, 128, 65), f)
    gvp[:, :G, :64] = gvv
    gvp[:, :G, 64] = 1.0
    gvp[:, :G] *= np.exp(gm)[:, :, None]
    gvp = gvp.astype(bf)

    return [
        {
            "qt": qt[c * SL : (c + 1) * SL],
            "lkt": lkt[c * SL : (c + 1) * SL],
            "skt": skt[c * SL : (c + 1) * SL],
            "gkt": gkt[c * SL : (c + 1) * SL],
            "lv": lvp[c * SL : (c + 1) * SL],
            "sv": svp[c * SL : (c + 1) * SL],
            "gv": gvp[c * SL : (c + 1) * SL],
        }
        for c in range(NCORES)
    ]


_NC_CACHE = {}
LAST_RESULTS = None


def kernel(**inputs):
    global LAST_RESULTS
    if "nc" not in _NC_CACHE:
        _NC_CACHE["nc"] = _build_bass()
    nc = _NC_CACHE["nc"]
    in_maps = _prepare(inputs)
    res = run_bass_kernel_spmd(nc, in_maps, core_ids=list(range(NCORES)))
    LAST_RESULTS = res
    out = np.empty((NH, T, D), np.float32)
    for c in range(NCORES):
        out[c * SL : (c + 1) * SL] = res.results[c]["o"]
    return out.reshape(N, H, T, D)


# revision 27
# speedup vs baseline: 1.1766x; 1.1766x over previous
"""Block-local sparse attention (LSG-style) on 8 TRN2 NeuronCores.

Sharding: the 32 (n, h) pairs are split 4-per-core (data/head parallel, no
collectives). Host-side numpy prep re-lays-out the inputs so the device
kernel needs no transposes, all bf16:

  - qt : Q^T per head [64, T]
  - lkt/skt/gkt: local/sparse/global K^T, token-padded with zeros
  - lv/sv/gv: V with a ones column appended (col 64), chunked [128, c, 65],
    and every row scaled by exp(mask): softmax(QK/8 + m) @ V is computed as
    sum_t exp(s_t) e^{m_t} [V_t, 1], then a divide by the accumulated last
    column — exact for any additive mask, and pad tokens (e^{m}=0) vanish
    from both numerator and denominator, so no mask row and no
    max-subtraction are needed (|QK|/8 is O(5), well within fp32 exp range).
    sv additionally holds 4 phase-shifted copies so the 32-token-granular
    sparse windows always start at partition 0.

The device processes query-block PAIRS: 9 score matmuls per pair into a
3-bank PSUM region [128, 1536] laid out so no matmul output crosses a
bank, exp(S/8) on ACT split in two ops (cols 0:1024 after 6 matmuls,
1024:1536 after all 9) so the PV matmuls can start on the sparse/global/
loc1 chunks while the local exp still runs, then 12 PV matmuls (N=65)
into [q, V|Z] and a per-row PSUM divide on DVE (no reciprocal RAW, no
drain).

Raw bass with hand-placed semaphores (walrus: at most one sem wait per
matmul/ACT instruction). Queue assignment keeps the stores from ever
queueing behind the multi-MB slot loads: input loads issue from the idle
GpSimd queue (K-side and V-side tensors on separate counting semaphores
so scores start after 1.26MB instead of 2.57MB), output stores (one
merged DMA per pair) from the Sync queue. Scores of pair p+2 issue right
after PV of pair p; optional filler matmuls (FILL x 512 cols into the
just-consumed psS bank, harmlessly overwritten by the next scores'
start=True) keep the PE gap-free so the DVFS p-state can ramp
1.2 GHz -> 2.4 GHz.
"""

from contextlib import ExitStack

import numpy as np

import concourse.bass as bass
import concourse.mybir as mybir
from concourse.bass_utils import run_bass_kernel_spmd

N, H, T, D = 2, 16, 4096, 64
B = 128          # query block
NB = T // B      # 32
G = 64           # global tokens
TSP = T // 4     # sparse tokens (1024)
NH = N * H       # 32
NCORES = 8
SL = NH // NCORES  # 4 heads per core
NP = SL * NB // 2  # 64 block-pairs per core
PPS = NB // 2      # 16 pairs per slot

LKT_W = T + 2 * B            # 4352 padded local tokens
SKT_W = TSP + 320            # 1344 padded sparse tokens
LV_C = LKT_W // 128          # 34 local V chunks
SV_C = 11                    # sparse V chunks per phase

F32 = mybir.dt.float32
BF16 = mybir.dt.bfloat16
GE = "sem-ge"

FILL = 0  # PE filler matmuls per pair (measured: no DVFS ramp on this HW)

# column layout of the per-pair score/prob tile [128, 1536] (3 PSUM banks;
# regions never cross a 512-col bank boundary).  exp op A covers [0:1024),
# exp op B covers [1024:1536).
C_SP1A, C_SP1B = 0, 128
C_SP2A, C_SP2B = 256, 384
C_G = 512        # 256 wide: q of both blocks
C_LOC1 = 768     # 256 wide: local chunk b+1, both blocks
C_LOC0 = 1024    # 128: local chunk b, block A only
C_LOC2 = 1152    # 256 wide: local chunk b+2, both blocks
C_LOC3 = 1408    # 128: local chunk b+3, block B only


def _build_bass():
    nc = bass.Bass("TRN2", num_devices=NCORES, debug=False)

    qt = nc.dram_tensor("qt", [SL, 64, T], BF16, kind="ExternalInput")
    lkt = nc.dram_tensor("lkt", [SL, 64, LKT_W], BF16, kind="ExternalInput")
    skt = nc.dram_tensor("skt", [SL, 64, SKT_W], BF16, kind="ExternalInput")
    gkt = nc.dram_tensor("gkt", [SL, 64, 128], BF16, kind="ExternalInput")
    lv = nc.dram_tensor("lv", [SL, 128, LV_C * 65], BF16, kind="ExternalInput")
    sv = nc.dram_tensor("sv", [SL, 128, 4 * SV_C * 65], BF16, kind="ExternalInput")
    gv = nc.dram_tensor("gv", [SL, 128, 65], BF16, kind="ExternalInput")
    o = nc.dram_tensor("o", [SL, T, D], F32, kind="ExternalOutput")

    EXP = mybir.ActivationFunctionType.Exp

    with ExitStack() as es:
        ec = es.enter_context
        # double-buffered inputs (slot parity).  K-side tensors are 128
        # partitions with rows 64:128 zeroed once at startup: keeping every
        # matmul at 128-partition contraction avoids the PE row-mode switch
        # (64p<->128p) that drains the array ~200ns at each phase boundary.
        qt_t = [ec(nc.sbuf_tensor(f"qt_t{i}", [128, T], BF16)) for i in range(2)]
        lkt_t = [ec(nc.sbuf_tensor(f"lkt_t{i}", [128, LKT_W], BF16)) for i in range(2)]
        skt_t = [ec(nc.sbuf_tensor(f"skt_t{i}", [128, SKT_W], BF16)) for i in range(2)]
        gkt_t = [ec(nc.sbuf_tensor(f"gkt_t{i}", [128, 128], BF16)) for i in range(2)]
        lv_t = [ec(nc.sbuf_tensor(f"lv_t{i}", [128, LV_C * 65], BF16)) for i in range(2)]
        sv_t = [ec(nc.sbuf_tensor(f"sv_t{i}", [128, 4 * SV_C * 65], BF16)) for i in range(2)]
        gv_t = [ec(nc.sbuf_tensor(f"gv_t{i}", [128, 65], BF16)) for i in range(2)]
        # double-buffered per-pair working set (pair parity)
        psS = [ec(nc.psum_tensor(f"psS{i}", [128, 1536], F32)) for i in range(2)]  # 3 banks
        pv = [ec(nc.psum_tensor(f"pv{i}", [128, 512], F32)) for i in range(2)]     # 1 bank
        pp = [ec(nc.sbuf_tensor(f"pp{i}", [128, 1536], BF16)) for i in range(2)]
        rec = [ec(nc.sbuf_tensor(f"rec{i}", [128, 2], F32)) for i in range(2)]
        # 8-deep output ring: slot-load DMA bursts delay store packets by up
        # to ~10us in the shared engines; 8 pairs of slack rides that out.
        OBN = 8
        ob = [ec(nc.sbuf_tensor(f"ob{i}", [128, 128], F32)) for i in range(OBN)]

        diK = [ec(nc.semaphore(f"diK{i}")) for i in range(2)]  # K-side loads, slot parity
        diV = [ec(nc.semaphore(f"diV{i}")) for i in range(2)]  # V-side loads, slot parity
        st = ec(nc.semaphore("st"))      # out stores (+16 per store, FIFO)
        iz = ec(nc.semaphore("iz"))      # K-tensor rows 64:128 zeroed
        pe_s = ec(nc.semaphore("pe_s"))  # +2 per pair: score matmuls (6 and 9) done
        pe_v = ec(nc.semaphore("pe_v"))  # +1 per pair: PV matmuls done
        act = ec(nc.semaphore("act"))    # +2 per pair: exp halves done
        dve = ec(nc.semaphore("dve"))    # +1 per pair: divide done
        block = ec(nc.Block(no_gpsimd_drain=True))

        @block.gpsimd
        def _(gpsimd):
            def load_slot(s, gate):
                u = s % 2
                kside = (
                    (skt_t[u], skt[s]),
                    (gkt_t[u], gkt[s]),
                    (qt_t[u], qt[s]),
                    (lkt_t[u], lkt[s]),
                )
                vside = (
                    (gv_t[u], gv[s]),
                    (sv_t[u], sv[s]),
                    (lv_t[u], lv[s]),
                )
                first = True
                for dst, src in kside:
                    d = gpsimd.dma_start(dst[0:64, :], src)
                    if first and gate is not None:
                        d.wait_op(pe_v, gate, GE)
                        first = False
                    d.then_inc(diK[u], 16)
                for dst, src in vside:
                    gpsimd.dma_start(dst[:], src).then_inc(diV[u], 16)

            load_slot(0, None)
            load_slot(1, None)
            load_slot(2, 16)
            load_slot(3, 32)

        def emit_scores(p, lo=0, hi=9):
            s, hb = divmod(p, PPS)
            b = 2 * hb
            su = s % 2
            if lo == 0 and hb == 0:
                nc.tensor.wait_ge(diK[su], 64 * (s // 2 + 1))
            qA = qt_t[su][:, b * B : (b + 1) * B]
            qB = qt_t[su][:, (b + 1) * B : (b + 2) * B]
            qAB = qt_t[su][:, b * B : (b + 2) * B]
            w1a, w2a = 32 * b, 32 * b + 224
            w1b, w2b = w1a + 32, w2a + 32
            u = p % 2
            mms = (
                (C_SP1A, 128, skt_t[su][:, w1a : w1a + 128], qA),
                (C_SP1B, 128, skt_t[su][:, w1b : w1b + 128], qB),
                (C_SP2A, 128, skt_t[su][:, w2a : w2a + 128], qA),
                (C_SP2B, 128, skt_t[su][:, w2b : w2b + 128], qB),
                (C_G, 256, gkt_t[su][:, :], qAB),
                (C_LOC1, 256, lkt_t[su][:, (b + 1) * B : (b + 2) * B], qAB),
                (C_LOC0, 128, lkt_t[su][:, b * B : (b + 1) * B], qA),
                (C_LOC2, 256, lkt_t[su][:, (b + 2) * B : (b + 3) * B], qAB),
                (C_LOC3, 128, lkt_t[su][:, (b + 3) * B : (b + 4) * B], qB),
            )
            for kk in range(lo, hi):
                col, w, lhsT, rhs = mms[kk]
                mm = nc.tensor.matmul(
                    psS[u][:, col : col + w],
                    lhsT, rhs,
                    start=True, stop=True,
                )
                if kk == 5 or kk == 8:
                    mm.then_inc(pe_s, 1)

        def pv_mms(p):
            s, hb = divmod(p, PPS)
            b = 2 * hb
            u = p % 2
            su = s % 2
            sp = []
            for blk in range(2):
                bb = b + blk
                w1, w2 = 32 * bb, 32 * bb + 224
                c1, r1 = divmod(w1, 128)
                c2, r2 = divmod(w2, 128)
                sp.append((((r1 // 32) * SV_C + c1) * 65, ((r2 // 32) * SV_C + c2) * 65))
            outA = pv[u][:, 0:65]
            outB = pv[u][:, 128:193]
            # Sequential accumulation groups (A fully, then B): a start=True
            # marks the surrounding 2KB PSUM zero-region pending-zero, so two
            # interleaved in-flight groups in one bank corrupt each other.
            # (out, pp col, rhs, start, stop)
            return (
                (outA, C_SP1A, sv_t[su][:, sp[0][0] : sp[0][0] + 65], True, False),
                (outA, C_SP2A, sv_t[su][:, sp[0][1] : sp[0][1] + 65], False, False),
                (outA, C_G, gv_t[su][:], False, False),
                (outA, C_LOC1, lv_t[su][:, (b + 1) * 65 : (b + 2) * 65], False, False),
                (outA, C_LOC0, lv_t[su][:, b * 65 : b * 65 + 65], False, False),
                (outA, C_LOC2, lv_t[su][:, (b + 2) * 65 : (b + 3) * 65], False, True),
                (outB, C_SP1B, sv_t[su][:, sp[1][0] : sp[1][0] + 65], True, False),
                (outB, C_SP2B, sv_t[su][:, sp[1][1] : sp[1][1] + 65], False, False),
                (outB, C_G + 128, gv_t[su][:], False, False),
                (outB, C_LOC1 + 128, lv_t[su][:, (b + 1) * 65 : (b + 2) * 65], False, False),
                (outB, C_LOC2 + 128, lv_t[su][:, (b + 2) * 65 : (b + 3) * 65], False, False),
                (outB, C_LOC3, lv_t[su][:, (b + 3) * 65 : (b + 4) * 65], False, True),
            )

        def emit_pv_range(p, mms, lo, hi):
            u = p % 2
            for kk in range(lo, hi):
                out, col, rhs, st_, sp_ = mms[kk]
                mm = nc.tensor.matmul(
                    out, pp[u][:, col : col + 128], rhs,
                    start=st_, stop=sp_, skip_group_check=True,
                )
                if kk == 11:
                    mm.then_inc(pe_v, 1)

        @block.tensor
        def _(tensor):
            tensor.wait_ge(iz, 1)
            tensor.wait_ge(diK[0], 64)
            emit_scores(0)
            emit_scores(1)
            for p in range(NP):
                s, hb = divmod(p, PPS)
                su = s % 2
                if p >= 2:
                    tensor.wait_ge(dve, p - 1)  # pv[u] free
                if hb == 0:
                    tensor.wait_ge(diV[su], 48 * (s // 2 + 1))
                mms = pv_mms(p)
                # PV chunks gated on exp of psS cols 0:1024, interleaved with
                # the next-next pair's score matmuls over the same free
                # columns so the PE stays busy while exp_b(p) finishes.
                tensor.wait_ge(act, 2 * p + 1)
                emit_pv_range(p, mms, 0, 4)   # A: sp1 sp2 G loc1
                if p + 2 < NP:
                    emit_scores(p + 2, 0, 6)  # sp x4, G, LOC1 (cols 0:1024)
                tensor.wait_ge(act, 2 * p + 2)
                emit_pv_range(p, mms, 4, 12)  # A: loc0 loc2; B: all
                if p + 2 < NP:
                    emit_scores(p + 2, 6, 9)  # LOC0 LOC2 LOC3 (cols 1024:)

        @block.scalar
        def _(scalar):
            # exp split 1024+512 (a long single op measures ~340ns slower
            # than the two split ops - PSUM bank-crossing penalty)
            for p in range(NP):
                u = p % 2
                if p >= 2:
                    scalar.wait_ge(pe_v, p - 1)  # pp[u] free: PV of p-2 done
                nc.scalar.activation(
                    pp[u][:, 0:1024], psS[u][:, 0:1024], EXP, scale=0.125
                ).wait_op(pe_s, 2 * p + 1, GE).then_inc(act, 1)
                nc.scalar.activation(
                    pp[u][:, 1024:1536], psS[u][:, 1024:1536], EXP, scale=0.125
                ).wait_op(pe_s, 2 * p + 2, GE).then_inc(act, 1)

        @block.vector
        def _(vector):
            # zero rows 64:128 of the K-side tensors once, overlapped with
            # the slot-0 DMA (DVE is idle until the first normalize anyway)
            for tt in (qt_t[0], qt_t[1], lkt_t[0], lkt_t[1],
                       skt_t[0], skt_t[1], gkt_t[0], gkt_t[1]):
                nc.vector.memzero(tt[64:128, :])
            nc.vector.drain()
            nc.vector.nop().then_inc(iz, 1)
            for p in range(NP):
                u = p % 2
                w = p % OBN
                if p >= OBN:
                    vector.wait_ge(st, 16 * (p - OBN + 1))  # ob[w] stored
                nc.vector.reciprocal(rec[u][:, 0:1], pv[u][:, 64:65]).wait_op(
                    pe_v, p + 1, GE
                )
                nc.vector.reciprocal(rec[u][:, 1:2], pv[u][:, 192:193])
                nc.vector.drain()  # DVE pipeline RAW: rec written, read next
                nc.vector.tensor_mul(
                    ob[w][:, 0:64], pv[u][:, 0:64],
                    rec[u][:, 0:1].broadcast_to([128, 64]),
                )
                nc.vector.tensor_mul(
                    ob[w][:, 64:128], pv[u][:, 128:192],
                    rec[u][:, 1:2].broadcast_to([128, 64]),
                ).then_inc(dve, 1)

        @block.sync
        def _(sync):
            for p in range(NP):
                s, hb = divmod(p, PPS)
                b = 2 * hb
                dst = o[s, b * B : (b + 2) * B, :].rearrange(
                    "(blk q) d -> q blk d", blk=2
                )
                src = ob[p % OBN][:, 0:128].rearrange("q (blk d) -> q blk d", blk=2)
                sync.dma_start(dst, src).wait_op(dve, p + 1, GE).then_inc(st, 16)
            sync.wait_ge(st, 16 * NP)

    return nc


def _prepare(inputs):
    import ml_dtypes

    bf = ml_dtypes.bfloat16
    f = np.float32
    q = np.asarray(inputs["query_layer"], f).reshape(NH, T, D)
    k = np.asarray(inputs["key_layer"], f).reshape(NH, T, D)
    v = np.asarray(inputs["value_layer"], f).reshape(NH, T, D)
    sk = np.asarray(inputs["sparse_key"], f).reshape(NH, TSP, D)
    svv = np.asarray(inputs["sparse_value"], f).reshape(NH, TSP, D)
    gk = np.asarray(inputs["global_key"], f).reshape(NH, G, D)
    gvv = np.asarray(inputs["global_value"], f).reshape(NH, G, D)
    am = np.repeat(np.asarray(inputs["attention_mask"], f)[:, 0, 0, :], H, 0)
    sm = np.repeat(np.asarray(inputs["sparse_mask"], f)[:, 0, 0, :], H, 0)
    gm = np.repeat(np.asarray(inputs["global_mask"], f)[:, 0, 0, :], H, 0)

    qt = np.ascontiguousarray(q.transpose(0, 2, 1)).astype(bf)

    lkt = np.zeros((NH, 64, LKT_W), f)
    lkt[:, :, B : B + T] = k.transpose(0, 2, 1)
    lkt = lkt.astype(bf)

    skt = np.zeros((NH, 64, SKT_W), f)
    skt[:, :, 160 : 160 + TSP] = sk.transpose(0, 2, 1)
    skt = skt.astype(bf)

    gkt = np.zeros((NH, 64, 128), f)
    gkt[:, :, :G] = gk.transpose(0, 2, 1)
    gkt = gkt.astype(bf)

    # V_aug rows scaled by exp(mask); pad rows are all-zero
    em_l = np.zeros((NH, LKT_W), f)
    em_l[:, B : B + T] = np.exp(am)
    lvp = np.zeros((NH, LKT_W, 65), f)
    lvp[:, B : B + T, :64] = v
    lvp[:, :, 64] = 1.0
    lvp *= em_l[:, :, None]
    lvp = np.ascontiguousarray(
        lvp.reshape(NH, LV_C, 128, 65).transpose(0, 2, 1, 3)
    ).reshape(NH, 128, LV_C * 65).astype(bf)

    SVP_W = 96 + SV_C * 128
    em_s = np.zeros((NH, SVP_W), f)
    em_s[:, 160 : 160 + TSP] = np.exp(sm)
    sv_pad = np.zeros((NH, SVP_W, 65), f)
    sv_pad[:, 160 : 160 + TSP, :64] = svv
    sv_pad[:, :, 64] = 1.0
    sv_pad *= em_s[:, :, None]
    svp = np.empty((NH, 4, 128, SV_C, 65), f)
    for p in range(4):
        svp[:, p] = (
            sv_pad[:, 32 * p : 32 * p + SV_C * 128]
            .reshape(NH, SV_C, 128, 65)
            .transpose(0, 2, 1, 3)
        )
    svp = np.ascontiguousarray(svp.transpose(0, 2, 1, 3, 4)).reshape(
        NH, 128, 4 * SV_C * 65
    ).astype(bf)

    gvp = np.zeros((NH, 128, 65), f)
    gvp[:, :G, :64] = gvv
    gvp[:, :G, 64] = 1.0
    gvp[:, :G] *= np.exp(gm)[:, :, None]
    gvp = gvp.astype(bf)

    return [
        {
            "qt": qt[c * SL : (c + 1) * SL],
            "lkt": lkt[c * SL : (c + 1) * SL],
            "skt": skt[c * SL : (c + 1) * SL],
            "gkt": gkt[c * SL : (c + 1) * SL],
            "lv": lvp[c * SL : (c + 1) * SL],
            "sv": svp[c * SL : (c + 1) * SL],
            "gv": gvp[c * SL : (c + 1) * SL],
        }
        for c in range(NCORES)
    ]


_NC_CACHE = {}
LAST_RESULTS = None


def kernel(**inputs):
    global LAST_RESULTS
    if "nc" not in _NC_CACHE:
        _NC_CACHE["nc"] = _build_bass()
    nc = _NC_CACHE["nc"]
    in_maps = _prepare(inputs)
    res = run_bass_kernel_spmd(nc, in_maps, core_ids=list(range(NCORES)))
    LAST_RESULTS = res
    out = np.empty((NH, T, D), np.float32)
    for c in range(NCORES):
        out[c * SL : (c + 1) * SL] = res.results[c]["o"]
    return out.reshape(N, H, T, D)


# revision 29
# speedup vs baseline: 1.1976x; 1.0178x over previous
"""Block-local sparse attention (LSG-style) on 8 TRN2 NeuronCores.

Sharding: the 32 (n, h) pairs are split 4-per-core (data/head parallel, no
collectives). Host-side numpy prep re-lays-out the inputs so the device
kernel needs no transposes, all bf16:

  - qt : Q^T per head [64, T]
  - lkt/skt/gkt: local/sparse/global K^T, token-padded with zeros
  - lv/sv/gv: V with a ones column appended (col 64), chunked [128, c, 65],
    and every row scaled by exp(mask): softmax(QK/8 + m) @ V is computed as
    sum_t exp(s_t) e^{m_t} [V_t, 1], then a divide by the accumulated last
    column — exact for any additive mask, and pad tokens (e^{m}=0) vanish
    from both numerator and denominator, so no mask row and no
    max-subtraction are needed (|QK|/8 is O(5), well within fp32 exp range).
    sv additionally holds 4 phase-shifted copies so the 32-token-granular
    sparse windows always start at partition 0.

The device processes query-block PAIRS: 9 score matmuls per pair into a
3-bank PSUM region [128, 1536] laid out so no matmul output crosses a
bank, exp(S/8) on ACT split in two ops (cols 0:1024 after 6 matmuls,
1024:1536 after all 9) so the PV matmuls can start on the sparse/global/
loc1 chunks while the local exp still runs, then 12 PV matmuls (N=65)
into [q, V|Z] and a per-row PSUM divide on DVE (no reciprocal RAW, no
drain).

Raw bass with hand-placed semaphores (walrus: at most one sem wait per
matmul/ACT instruction). Queue assignment keeps the stores from ever
queueing behind the multi-MB slot loads: input loads issue from the idle
GpSimd queue (K-side and V-side tensors on separate counting semaphores
so scores start after 1.26MB instead of 2.57MB), output stores (one
merged DMA per pair) from the Sync queue. Scores of pair p+2 issue right
after PV of pair p; optional filler matmuls (FILL x 512 cols into the
just-consumed psS bank, harmlessly overwritten by the next scores'
start=True) keep the PE gap-free so the DVFS p-state can ramp
1.2 GHz -> 2.4 GHz.
"""

from contextlib import ExitStack

import numpy as np

import concourse.bass as bass
import concourse.mybir as mybir
from concourse.bass_utils import run_bass_kernel_spmd

N, H, T, D = 2, 16, 4096, 64
B = 128          # query block
NB = T // B      # 32
G = 64           # global tokens
TSP = T // 4     # sparse tokens (1024)
NH = N * H       # 32
NCORES = 8
SL = NH // NCORES  # 4 heads per core
NP = SL * NB // 2  # 64 block-pairs per core
PPS = NB // 2      # 16 pairs per slot

LKT_W = T + 2 * B            # 4352 padded local tokens
SKT_W = TSP + 320            # 1344 padded sparse tokens
LV_C = LKT_W // 128          # 34 local V chunks
SV_C = 11                    # sparse V chunks per phase

F32 = mybir.dt.float32
BF16 = mybir.dt.bfloat16
GE = "sem-ge"

FILL = 0  # PE filler matmuls per pair (measured: no DVFS ramp on this HW)

# column layout of the per-pair score/prob tile [128, 1536] (3 PSUM banks;
# regions never cross a 512-col bank boundary).  exp op A covers [0:1024),
# exp op B covers [1024:1536).
C_SP1A, C_SP1B = 0, 128
C_SP2A, C_SP2B = 256, 384
C_G = 512        # 256 wide: q of both blocks
C_LOC1 = 768     # 256 wide: local chunk b+1, both blocks
C_LOC0 = 1024    # 128: local chunk b, block A only
C_LOC2 = 1152    # 256 wide: local chunk b+2, both blocks
C_LOC3 = 1408    # 128: local chunk b+3, block B only


def _build_bass():
    nc = bass.Bass("TRN2", num_devices=NCORES, debug=False)

    qt = nc.dram_tensor("qt", [SL, 64, T], BF16, kind="ExternalInput")
    lkt = nc.dram_tensor("lkt", [SL, 64, LKT_W], BF16, kind="ExternalInput")
    skt = nc.dram_tensor("skt", [SL, 64, SKT_W], BF16, kind="ExternalInput")
    gkt = nc.dram_tensor("gkt", [SL, 64, 128], BF16, kind="ExternalInput")
    lv = nc.dram_tensor("lv", [SL, 128, LV_C * 65], BF16, kind="ExternalInput")
    sv = nc.dram_tensor("sv", [SL, 128, 4 * SV_C * 65], BF16, kind="ExternalInput")
    gv = nc.dram_tensor("gv", [SL, 128, 65], BF16, kind="ExternalInput")
    o = nc.dram_tensor("o", [SL, T, D], F32, kind="ExternalOutput")

    EXP = mybir.ActivationFunctionType.Exp

    with ExitStack() as es:
        ec = es.enter_context
        # double-buffered inputs (slot parity).  K-side tensors are 128
        # partitions with rows 64:128 zeroed once at startup: keeping every
        # matmul at 128-partition contraction avoids the PE row-mode switch
        # (64p<->128p) that drains the array ~200ns at each phase boundary.
        qt_t = [ec(nc.sbuf_tensor(f"qt_t{i}", [128, T], BF16)) for i in range(2)]
        lkt_t = [ec(nc.sbuf_tensor(f"lkt_t{i}", [128, LKT_W], BF16)) for i in range(2)]
        skt_t = [ec(nc.sbuf_tensor(f"skt_t{i}", [128, SKT_W], BF16)) for i in range(2)]
        gkt_t = [ec(nc.sbuf_tensor(f"gkt_t{i}", [128, 128], BF16)) for i in range(2)]
        lv_t = [ec(nc.sbuf_tensor(f"lv_t{i}", [128, LV_C * 65], BF16)) for i in range(2)]
        sv_t = [ec(nc.sbuf_tensor(f"sv_t{i}", [128, 4 * SV_C * 65], BF16)) for i in range(2)]
        gv_t = [ec(nc.sbuf_tensor(f"gv_t{i}", [128, 65], BF16)) for i in range(2)]
        # double-buffered per-pair working set (pair parity)
        psS = [ec(nc.psum_tensor(f"psS{i}", [128, 1536], F32)) for i in range(2)]  # 3 banks
        pv = [ec(nc.psum_tensor(f"pv{i}", [128, 512], F32)) for i in range(2)]     # 1 bank
        pp = [ec(nc.sbuf_tensor(f"pp{i}", [128, 1536], BF16)) for i in range(2)]
        rec = [ec(nc.sbuf_tensor(f"rec{i}", [128, 2], F32)) for i in range(2)]
        # 8-deep output ring: slot-load DMA bursts delay store packets by up
        # to ~10us in the shared engines; 8 pairs of slack rides that out.
        OBN = 8
        ob = [ec(nc.sbuf_tensor(f"ob{i}", [128, 128], F32)) for i in range(OBN)]

        diK = [ec(nc.semaphore(f"diK{i}")) for i in range(2)]  # K-side loads, slot parity
        diV = [ec(nc.semaphore(f"diV{i}")) for i in range(2)]  # V-side loads, slot parity
        st = ec(nc.semaphore("st"))      # out stores (+16 per store, FIFO)
        iz = ec(nc.semaphore("iz"))      # K-tensor rows 64:128 zeroed
        pe_s = ec(nc.semaphore("pe_s"))  # +2 per pair: score matmuls (6 and 9) done
        pe_v = ec(nc.semaphore("pe_v"))  # +1 per pair: PV matmuls done
        act = ec(nc.semaphore("act"))    # +2 per pair: exp halves done
        dve = ec(nc.semaphore("dve"))    # +1 per pair: divide done
        block = ec(nc.Block(no_gpsimd_drain=True))

        @block.gpsimd
        def _(gpsimd):
            def load_slot(s, gate):
                u = s % 2
                kside = (
                    (skt_t[u], skt[s]),
                    (gkt_t[u], gkt[s]),
                    (qt_t[u], qt[s]),
                    (lkt_t[u], lkt[s]),
                )
                vside = (
                    (gv_t[u], gv[s]),
                    (sv_t[u], sv[s]),
                    (lv_t[u], lv[s]),
                )
                first = True
                for dst, src in kside:
                    d = gpsimd.dma_start(dst[0:64, :], src)
                    if first and gate is not None:
                        d.wait_op(pe_v, gate, GE)
                        first = False
                    d.then_inc(diK[u], 16)
                for dst, src in vside:
                    gpsimd.dma_start(dst[:], src).then_inc(diV[u], 16)

            load_slot(0, None)
            load_slot(1, None)
            load_slot(2, 16)
            load_slot(3, 32)

        def emit_scores(p, lo=0, hi=9):
            s, hb = divmod(p, PPS)
            b = 2 * hb
            su = s % 2
            if lo == 0 and hb == 0:
                nc.tensor.wait_ge(diK[su], 64 * (s // 2 + 1))
            qA = qt_t[su][:, b * B : (b + 1) * B]
            qB = qt_t[su][:, (b + 1) * B : (b + 2) * B]
            qAB = qt_t[su][:, b * B : (b + 2) * B]
            w1a, w2a = 32 * b, 32 * b + 224
            w1b, w2b = w1a + 32, w2a + 32
            u = p % 2
            mms = (
                (C_SP1A, 128, skt_t[su][:, w1a : w1a + 128], qA),
                (C_SP1B, 128, skt_t[su][:, w1b : w1b + 128], qB),
                (C_SP2A, 128, skt_t[su][:, w2a : w2a + 128], qA),
                (C_SP2B, 128, skt_t[su][:, w2b : w2b + 128], qB),
                (C_G, 256, gkt_t[su][:, :], qAB),
                (C_LOC1, 256, lkt_t[su][:, (b + 1) * B : (b + 2) * B], qAB),
                (C_LOC0, 128, lkt_t[su][:, b * B : (b + 1) * B], qA),
                (C_LOC2, 256, lkt_t[su][:, (b + 2) * B : (b + 3) * B], qAB),
                (C_LOC3, 128, lkt_t[su][:, (b + 3) * B : (b + 4) * B], qB),
            )
            for kk in range(lo, hi):
                col, w, lhsT, rhs = mms[kk]
                mm = nc.tensor.matmul(
                    psS[u][:, col : col + w],
                    lhsT, rhs,
                    start=True, stop=True,
                )
                if kk == 5 or kk == 8:
                    mm.then_inc(pe_s, 1)

        def pv_mms(p):
            s, hb = divmod(p, PPS)
            b = 2 * hb
            u = p % 2
            su = s % 2
            sp = []
            for blk in range(2):
                bb = b + blk
                w1, w2 = 32 * bb, 32 * bb + 224
                c1, r1 = divmod(w1, 128)
                c2, r2 = divmod(w2, 128)
                sp.append((((r1 // 32) * SV_C + c1) * 65, ((r2 // 32) * SV_C + c2) * 65))
            outA = pv[u][:, 0:65]
            outB = pv[u][:, 128:193]
            # Sequential accumulation groups (A fully, then B): a start=True
            # marks the surrounding 2KB PSUM zero-region pending-zero, so two
            # interleaved in-flight groups in one bank corrupt each other.
            # (out, pp col, rhs, start, stop)
            return (
                (outA, C_SP1A, sv_t[su][:, sp[0][0] : sp[0][0] + 65], True, False),
                (outA, C_SP2A, sv_t[su][:, sp[0][1] : sp[0][1] + 65], False, False),
                (outA, C_G, gv_t[su][:], False, False),
                (outA, C_LOC1, lv_t[su][:, (b + 1) * 65 : (b + 2) * 65], False, False),
                (outA, C_LOC0, lv_t[su][:, b * 65 : b * 65 + 65], False, False),
                (outA, C_LOC2, lv_t[su][:, (b + 2) * 65 : (b + 3) * 65], False, True),
                (outB, C_SP1B, sv_t[su][:, sp[1][0] : sp[1][0] + 65], True, False),
                (outB, C_SP2B, sv_t[su][:, sp[1][1] : sp[1][1] + 65], False, False),
                (outB, C_G + 128, gv_t[su][:], False, False),
                (outB, C_LOC1 + 128, lv_t[su][:, (b + 1) * 65 : (b + 2) * 65], False, False),
                (outB, C_LOC2 + 128, lv_t[su][:, (b + 2) * 65 : (b + 3) * 65], False, False),
                (outB, C_LOC3, lv_t[su][:, (b + 3) * 65 : (b + 4) * 65], False, True),
            )

        def emit_pv_range(p, mms, lo, hi):
            u = p % 2
            for kk in range(lo, hi):
                out, col, rhs, st_, sp_ = mms[kk]
                mm = nc.tensor.matmul(
                    out, pp[u][:, col : col + 128], rhs,
                    start=st_, stop=sp_, skip_group_check=True,
                )
                if kk == 11:
                    mm.then_inc(pe_v, 1)

        @block.tensor
        def _(tensor):
            tensor.wait_ge(iz, 1)
            tensor.wait_ge(diK[0], 64)
            emit_scores(0)
            emit_scores(1)
            for p in range(NP):
                s, hb = divmod(p, PPS)
                su = s % 2
                if p >= 2:
                    tensor.wait_ge(dve, p - 1)  # pv[u] free
                if hb == 0:
                    tensor.wait_ge(diV[su], 48 * (s // 2 + 1))
                mms = pv_mms(p)
                # everything below needs only exp(p) done; scores(p+2) are
                # interleaved so pe_s fires mid-iteration, keeping ACT fed
                tensor.wait_ge(act, p + 1)
                emit_pv_range(p, mms, 0, 4)   # A: sp1 sp2 G loc1
                if p + 2 < NP:
                    emit_scores(p + 2, 0, 6)  # sp x4, G, LOC1
                emit_pv_range(p, mms, 4, 12)  # A: loc0 loc2; B: all
                if p + 2 < NP:
                    emit_scores(p + 2, 6, 9)  # LOC0 LOC2 LOC3

        @block.scalar
        def _(scalar):
            # one exp per pair: ACT is the pacer and each ACTIVATE pays a
            # 143ns PSUM-access bubble, so a single op is cheapest.  The PE
            # side stays busy because scores(p+2) interleave into PV(p).
            for p in range(NP):
                u = p % 2
                if p >= 2:
                    scalar.wait_ge(pe_v, p - 1)  # pp[u] free: PV of p-2 done
                nc.scalar.activation(
                    pp[u][:, 0:1536], psS[u][:, 0:1536], EXP, scale=0.125
                ).wait_op(pe_s, 2 * p + 2, GE).then_inc(act, 1)

        @block.vector
        def _(vector):
            # zero rows 64:128 of the K-side tensors once, overlapped with
            # the slot-0 DMA (DVE is idle until the first normalize anyway)
            for tt in (qt_t[0], qt_t[1], lkt_t[0], lkt_t[1],
                       skt_t[0], skt_t[1], gkt_t[0], gkt_t[1]):
                nc.vector.memzero(tt[64:128, :])
            nc.vector.drain()
            nc.vector.nop().then_inc(iz, 1)
            for p in range(NP):
                u = p % 2
                w = p % OBN
                if p >= OBN:
                    vector.wait_ge(st, 16 * (p - OBN + 1))  # ob[w] stored
                nc.vector.reciprocal(rec[u][:, 0:1], pv[u][:, 64:65]).wait_op(
                    pe_v, p + 1, GE
                )
                nc.vector.reciprocal(rec[u][:, 1:2], pv[u][:, 192:193])
                nc.vector.drain()  # DVE pipeline RAW: rec written, read next
                nc.vector.tensor_mul(
                    ob[w][:, 0:64], pv[u][:, 0:64],
                    rec[u][:, 0:1].broadcast_to([128, 64]),
                )
                nc.vector.tensor_mul(
                    ob[w][:, 64:128], pv[u][:, 128:192],
                    rec[u][:, 1:2].broadcast_to([128, 64]),
                ).then_inc(dve, 1)

        @block.sync
        def _(sync):
            for p in range(NP):
                s, hb = divmod(p, PPS)
                b = 2 * hb
                dst = o[s, b * B : (b + 2) * B, :].rearrange(
                    "(blk q) d -> q blk d", blk=2
                )
                src = ob[p % OBN][:, 0:128].rearrange("q (blk d) -> q blk d", blk=2)
                sync.dma_start(dst, src).wait_op(dve, p + 1, GE).then_inc(st, 16)
            sync.wait_ge(st, 16 * NP)

    return nc


def _prepare(inputs):
    import ml_dtypes

    bf = ml_dtypes.bfloat16
    f = np.float32
    q = np.asarray(inputs["query_layer"], f).reshape(NH, T, D)
    k = np.asarray(inputs["key_layer"], f).reshape(NH, T, D)
    v = np.asarray(inputs["value_layer"], f).reshape(NH, T, D)
    sk = np.asarray(inputs["sparse_key"], f).reshape(NH, TSP, D)
    svv = np.asarray(inputs["sparse_value"], f).reshape(NH, TSP, D)
    gk = np.asarray(inputs["global_key"], f).reshape(NH, G, D)
    gvv = np.asarray(inputs["global_value"], f).reshape(NH, G, D)
    am = np.repeat(np.asarray(inputs["attention_mask"], f)[:, 0, 0, :], H, 0)
    sm = np.repeat(np.asarray(inputs["sparse_mask"], f)[:, 0, 0, :], H, 0)
    gm = np.repeat(np.asarray(inputs["global_mask"], f)[:, 0, 0, :], H, 0)

    qt = np.ascontiguousarray(q.transpose(0, 2, 1)).astype(bf)

    lkt = np.zeros((NH, 64, LKT_W), f)
    lkt[:, :, B : B + T] = k.transpose(0, 2, 1)
    lkt = lkt.astype(bf)

    skt = np.zeros((NH, 64, SKT_W), f)
    skt[:, :, 160 : 160 + TSP] = sk.transpose(0, 2, 1)
    skt = skt.astype(bf)

    gkt = np.zeros((NH, 64, 128), f)
    gkt[:, :, :G] = gk.transpose(0, 2, 1)
    gkt = gkt.astype(bf)

    # V_aug rows scaled by exp(mask); pad rows are all-zero
    em_l = np.zeros((NH, LKT_W), f)
    em_l[:, B : B + T] = np.exp(am)
    lvp = np.zeros((NH, LKT_W, 65), f)
    lvp[:, B : B + T, :64] = v
    lvp[:, :, 64] = 1.0
    lvp *= em_l[:, :, None]
    lvp = np.ascontiguousarray(
        lvp.reshape(NH, LV_C, 128, 65).transpose(0, 2, 1, 3)
    ).reshape(NH, 128, LV_C * 65).astype(bf)

    SVP_W = 96 + SV_C * 128
    em_s = np.zeros((NH, SVP_W), f)
    em_s[:, 160 : 160 + TSP] = np.exp(sm)
    sv_pad = np.zeros((NH, SVP_W, 65), f)
    sv_pad[:, 160 : 160 + TSP, :64] = svv
    sv_pad[:, :, 64] = 1.0
    sv_pad *= em_s[:, :, None]
    svp = np.empty((NH, 4, 128, SV_C, 65), f)
    for p in range(4):
        svp[:, p] = (
            sv_pad[:, 32 * p : 32 * p + SV_C * 128]
            .reshape(NH, SV_C, 128, 65)
            .transpose(0, 2, 1, 3)
        )
    svp = np.ascontiguousarray(svp.transpose(0, 2, 1, 3, 4)).reshape(
        NH, 128, 4 * SV_C * 65
    ).astype(bf)

    gvp = np.zeros((NH, 128, 65), f)
    gvp[:, :G, :64] = gvv
    gvp[:, :G, 64] = 1.0
    gvp[:, :G] *= np.exp(gm)[:, :, None]
    gvp = gvp.astype(bf)

    return [
        {
            "qt": qt[c * SL : (c + 1) * SL],
            "lkt": lkt[c * SL : (c + 1) * SL],
            "skt": skt[c * SL : (c + 1) * SL],
            "gkt": gkt[c * SL : (c + 1) * SL],
            "lv": lvp[c * SL : (c + 1) * SL],
            "sv": svp[c * SL : (c + 1) * SL],
            "gv": gvp[c * SL : (c + 1) * SL],
        }
        for c in range(NCORES)
    ]


_NC_CACHE = {}
LAST_RESULTS = None


def kernel(**inputs):
    global LAST_RESULTS
    if "nc" not in _NC_CACHE:
        _NC_CACHE["nc"] = _build_bass()
    nc = _NC_CACHE["nc"]
    in_maps = _prepare(inputs)
    res = run_bass_kernel_spmd(nc, in_maps, core_ids=list(range(NCORES)))
    LAST_RESULTS = res
    out = np.empty((NH, T, D), np.float32)
    for c in range(NCORES):
        out[c * SL : (c + 1) * SL] = res.results[c]["o"]
    return out.reshape(N, H, T, D)


# revision 35
# speedup vs baseline: 1.2103x; 1.0106x over previous
"""Block-local sparse attention (LSG-style) on 8 TRN2 NeuronCores.

Sharding: the 32 (n, h) pairs are split 4-per-core (data/head parallel, no
collectives). Host-side numpy prep re-lays-out the inputs so the device
kernel needs no transposes, all bf16:

  - qt : Q^T per head [64, T]
  - lkt/skt/gkt: local/sparse/global K^T, token-padded with zeros
  - lv/sv/gv: V with a ones column appended (col 64), chunked [128, c, 65],
    and every row scaled by exp(mask): softmax(QK/8 + m) @ V is computed as
    sum_t exp(s_t) e^{m_t} [V_t, 1], then a divide by the accumulated last
    column — exact for any additive mask, and pad tokens (e^{m}=0) vanish
    from both numerator and denominator, so no mask row and no
    max-subtraction are needed (|QK|/8 is O(5), well within fp32 exp range).
    sv additionally holds 4 phase-shifted copies so the 32-token-granular
    sparse windows always start at partition 0.

The device processes query-block PAIRS: 9 score matmuls per pair into a
3-bank PSUM region [128, 1536] laid out so no matmul output crosses a
bank, exp(S/8) on ACT split in two ops (cols 0:1024 after 6 matmuls,
1024:1536 after all 9) so the PV matmuls can start on the sparse/global/
loc1 chunks while the local exp still runs, then 12 PV matmuls (N=65)
into [q, V|Z] and a per-row PSUM divide on DVE (no reciprocal RAW, no
drain).

Raw bass with hand-placed semaphores (walrus: at most one sem wait per
matmul/ACT instruction). Queue assignment keeps the stores from ever
queueing behind the multi-MB slot loads: input loads issue from the idle
GpSimd queue (K-side and V-side tensors on separate counting semaphores
so scores start after 1.26MB instead of 2.57MB), output stores (one
merged DMA per pair) from the Sync queue. Scores of pair p+2 issue right
after PV of pair p; optional filler matmuls (FILL x 512 cols into the
just-consumed psS bank, harmlessly overwritten by the next scores'
start=True) keep the PE gap-free so the DVFS p-state can ramp
1.2 GHz -> 2.4 GHz.
"""

from contextlib import ExitStack

import numpy as np

import concourse.bass as bass
import concourse.mybir as mybir
from concourse.bass_utils import run_bass_kernel_spmd

N, H, T, D = 2, 16, 4096, 64
B = 128          # query block
NB = T // B      # 32
G = 64           # global tokens
TSP = T // 4     # sparse tokens (1024)
NH = N * H       # 32
NCORES = 8
SL = NH // NCORES  # 4 heads per core
NP = SL * NB // 2  # 64 block-pairs per core
PPS = NB // 2      # 16 pairs per slot

LKT_W = T + 2 * B            # 4352 padded local tokens
SKT_W = TSP + 320            # 1344 padded sparse tokens
LV_C = LKT_W // 128          # 34 local V chunks
SV_C = 11                    # sparse V chunks per phase

F32 = mybir.dt.float32
BF16 = mybir.dt.bfloat16
GE = "sem-ge"

FILL = 0  # PE filler matmuls per pair (measured: no DVFS ramp on this HW)

# column layout of the per-pair score/prob tile [128, 1536] (3 PSUM banks;
# regions never cross a 512-col bank boundary).  exp op A covers [0:1024),
# exp op B covers [1024:1536).
C_SP1A, C_SP1B = 0, 128
C_SP2A, C_SP2B = 256, 384
C_G = 512        # 256 wide: q of both blocks
C_LOC1 = 768     # 256 wide: local chunk b+1, both blocks
C_LOC0 = 1024    # 128: local chunk b, block A only
C_LOC2 = 1152    # 256 wide: local chunk b+2, both blocks
C_LOC3 = 1408    # 128: local chunk b+3, block B only


def _build_bass():
    nc = bass.Bass("TRN2", num_devices=NCORES, debug=False)

    qt = nc.dram_tensor("qt", [SL, 64, T], BF16, kind="ExternalInput")
    lkt = nc.dram_tensor("lkt", [SL, 64, LKT_W], BF16, kind="ExternalInput")
    skt = nc.dram_tensor("skt", [SL, 64, SKT_W], BF16, kind="ExternalInput")
    gkt = nc.dram_tensor("gkt", [SL, 64, 128], BF16, kind="ExternalInput")
    lv = nc.dram_tensor("lv", [SL, 128, LV_C * 65], BF16, kind="ExternalInput")
    sv = nc.dram_tensor("sv", [SL, 128, 4 * SV_C * 65], BF16, kind="ExternalInput")
    gv = nc.dram_tensor("gv", [SL, 128, 65], BF16, kind="ExternalInput")
    o = nc.dram_tensor("o", [SL, T, D], F32, kind="ExternalOutput")

    EXP = mybir.ActivationFunctionType.Exp

    with ExitStack() as es:
        ec = es.enter_context
        # double-buffered inputs (slot parity).  K-side tensors are 128
        # partitions with rows 64:128 zeroed once at startup: keeping every
        # matmul at 128-partition contraction avoids the PE row-mode switch
        # (64p<->128p) that drains the array ~200ns at each phase boundary.
        qt_t = [ec(nc.sbuf_tensor(f"qt_t{i}", [128, T], BF16)) for i in range(2)]
        lkt_t = [ec(nc.sbuf_tensor(f"lkt_t{i}", [128, LKT_W], BF16)) for i in range(2)]
        skt_t = [ec(nc.sbuf_tensor(f"skt_t{i}", [128, SKT_W], BF16)) for i in range(2)]
        gkt_t = [ec(nc.sbuf_tensor(f"gkt_t{i}", [128, 128], BF16)) for i in range(2)]
        lv_t = [ec(nc.sbuf_tensor(f"lv_t{i}", [128, LV_C * 65], BF16)) for i in range(2)]
        sv_t = [ec(nc.sbuf_tensor(f"sv_t{i}", [128, 4 * SV_C * 65], BF16)) for i in range(2)]
        gv_t = [ec(nc.sbuf_tensor(f"gv_t{i}", [128, 65], BF16)) for i in range(2)]
        # double-buffered per-pair working set (pair parity)
        psS = [ec(nc.psum_tensor(f"psS{i}", [128, 1536], F32)) for i in range(2)]  # 3 banks
        pv = [ec(nc.psum_tensor(f"pv{i}", [128, 512], F32)) for i in range(2)]     # 1 bank
        pp = [ec(nc.sbuf_tensor(f"pp{i}", [128, 1536], BF16)) for i in range(2)]
        rec = [ec(nc.sbuf_tensor(f"rec{i}", [128, 2], F32)) for i in range(2)]
        # 8-deep output ring: slot-load DMA bursts delay store packets by up
        # to ~10us in the shared engines; 8 pairs of slack rides that out.
        OBN = 8
        ob = [ec(nc.sbuf_tensor(f"ob{i}", [128, 128], F32)) for i in range(OBN)]

        diK = [ec(nc.semaphore(f"diK{i}")) for i in range(2)]  # K-side loads, slot parity
        diV = [ec(nc.semaphore(f"diV{i}")) for i in range(2)]  # V-side loads, slot parity
        st = ec(nc.semaphore("st"))      # out stores (+16 per store, FIFO)
        iz = ec(nc.semaphore("iz"))      # K-tensor rows 64:128 zeroed
        diP = ec(nc.semaphore("diP"))    # slot-0 V prefix (pairs 0-3)
        pe_s = ec(nc.semaphore("pe_s"))  # +2 per pair: score matmuls (6 and 9) done
        pe_v = ec(nc.semaphore("pe_v"))  # +1 per pair: PV matmuls done
        act = ec(nc.semaphore("act"))    # +2 per pair: exp halves done
        dve = ec(nc.semaphore("dve"))    # +1 per pair: divide done
        block = ec(nc.Block(no_gpsimd_drain=True))

        @block.gpsimd
        def _(gpsimd):
            def load_slot(s, gate):
                u = s % 2
                kside = (
                    (skt_t[u], skt[s]),
                    (gkt_t[u], gkt[s]),
                    (qt_t[u], qt[s]),
                    (lkt_t[u], lkt[s]),
                )
                vside = (
                    (gv_t[u], gv[s]),
                    (sv_t[u], sv[s]),
                    (lv_t[u], lv[s]),
                )
                first = True
                for dst, src in kside:
                    d = gpsimd.dma_start(dst[0:64, :], src)
                    if first and gate is not None:
                        d.wait_op(pe_v, gate, GE)
                        first = False
                    d.then_inc(diK[u], 16)
                for dst, src in vside:
                    gpsimd.dma_start(dst[:], src).then_inc(diV[u], 16)

            # slot 0 by hand: V prefix (sv chunks 0-3, lv chunks 0-9) covers
            # pairs 0-3 and gates PV(0) ~4.5us earlier than the full V side
            SVP = 4 * 4 * 65   # sv prefix cols (chunk-major)
            LVP = 10 * 65      # lv prefix cols
            for dst, src in (
                (skt_t[0], skt[0]),
                (gkt_t[0], gkt[0]),
                (qt_t[0], qt[0]),
                (lkt_t[0], lkt[0]),
            ):
                gpsimd.dma_start(dst[0:64, :], src).then_inc(diK[0], 16)
            gpsimd.dma_start(gv_t[0][:], gv[0]).then_inc(diP, 16)
            gpsimd.dma_start(sv_t[0][:, 0:SVP], sv[0][:, 0:SVP]).then_inc(diP, 16)
            gpsimd.dma_start(lv_t[0][:, 0:LVP], lv[0][:, 0:LVP]).then_inc(diP, 16)
            gpsimd.dma_start(
                sv_t[0][:, SVP : 4 * SV_C * 65], sv[0][:, SVP : 4 * SV_C * 65]
            ).then_inc(diV[0], 16)
            gpsimd.dma_start(
                lv_t[0][:, LVP : LV_C * 65], lv[0][:, LVP : LV_C * 65]
            ).then_inc(diV[0], 16)
            load_slot(1, None)
            load_slot(2, 16)
            load_slot(3, 32)

        def emit_scores(p, lo=0, hi=9):
            s, hb = divmod(p, PPS)
            b = 2 * hb
            su = s % 2
            if lo == 0 and hb == 0:
                nc.tensor.wait_ge(diK[su], 64 * (s // 2 + 1))
            qA = qt_t[su][:, b * B : (b + 1) * B]
            qB = qt_t[su][:, (b + 1) * B : (b + 2) * B]
            qAB = qt_t[su][:, b * B : (b + 2) * B]
            w1a, w2a = 32 * b, 32 * b + 224
            w1b, w2b = w1a + 32, w2a + 32
            u = p % 2
            mms = (
                (C_SP1A, 128, skt_t[su][:, w1a : w1a + 128], qA),
                (C_SP1B, 128, skt_t[su][:, w1b : w1b + 128], qB),
                (C_SP2A, 128, skt_t[su][:, w2a : w2a + 128], qA),
                (C_SP2B, 128, skt_t[su][:, w2b : w2b + 128], qB),
                (C_G, 256, gkt_t[su][:, :], qAB),
                (C_LOC1, 256, lkt_t[su][:, (b + 1) * B : (b + 2) * B], qAB),
                (C_LOC0, 128, lkt_t[su][:, b * B : (b + 1) * B], qA),
                (C_LOC2, 256, lkt_t[su][:, (b + 2) * B : (b + 3) * B], qAB),
                (C_LOC3, 128, lkt_t[su][:, (b + 3) * B : (b + 4) * B], qB),
            )
            for kk in range(lo, hi):
                col, w, lhsT, rhs = mms[kk]
                mm = nc.tensor.matmul(
                    psS[u][:, col : col + w],
                    lhsT, rhs,
                    start=True, stop=True,
                )
                if kk == 5 or kk == 8:
                    mm.then_inc(pe_s, 1)

        def pv_mms(p):
            s, hb = divmod(p, PPS)
            b = 2 * hb
            u = p % 2
            su = s % 2
            sp = []
            for blk in range(2):
                bb = b + blk
                w1, w2 = 32 * bb, 32 * bb + 224
                c1, r1 = divmod(w1, 128)
                c2, r2 = divmod(w2, 128)
                sp.append(((c1 * 4 + r1 // 32) * 65, (c2 * 4 + r2 // 32) * 65))
            outA = pv[u][:, 0:65]
            outB = pv[u][:, 128:193]
            # Sequential accumulation groups (A fully, then B): a start=True
            # marks the surrounding 2KB PSUM zero-region pending-zero, so two
            # interleaved in-flight groups in one bank corrupt each other.
            # (out, pp col, rhs, start, stop)
            return (
                (outA, C_SP1A, sv_t[su][:, sp[0][0] : sp[0][0] + 65], True, False),
                (outA, C_SP2A, sv_t[su][:, sp[0][1] : sp[0][1] + 65], False, False),
                (outA, C_G, gv_t[su][:], False, False),
                (outA, C_LOC1, lv_t[su][:, (b + 1) * 65 : (b + 2) * 65], False, False),
                (outA, C_LOC0, lv_t[su][:, b * 65 : b * 65 + 65], False, False),
                (outA, C_LOC2, lv_t[su][:, (b + 2) * 65 : (b + 3) * 65], False, True),
                (outB, C_SP1B, sv_t[su][:, sp[1][0] : sp[1][0] + 65], True, False),
                (outB, C_SP2B, sv_t[su][:, sp[1][1] : sp[1][1] + 65], False, False),
                (outB, C_G + 128, gv_t[su][:], False, False),
                (outB, C_LOC1 + 128, lv_t[su][:, (b + 1) * 65 : (b + 2) * 65], False, False),
                (outB, C_LOC2 + 128, lv_t[su][:, (b + 2) * 65 : (b + 3) * 65], False, False),
                (outB, C_LOC3, lv_t[su][:, (b + 3) * 65 : (b + 4) * 65], False, True),
            )

        def emit_pv_range(p, mms, lo, hi):
            u = p % 2
            for kk in range(lo, hi):
                out, col, rhs, st_, sp_ = mms[kk]
                mm = nc.tensor.matmul(
                    out, pp[u][:, col : col + 128], rhs,
                    start=st_, stop=sp_, skip_group_check=True,
                )
                if kk == 11:
                    mm.then_inc(pe_v, 1)

        @block.tensor
        def _(tensor):
            tensor.wait_ge(iz, 1)
            tensor.wait_ge(diK[0], 64)
            emit_scores(0)
            emit_scores(1)
            for p in range(NP):
                s, hb = divmod(p, PPS)
                su = s % 2
                if p >= 2:
                    tensor.wait_ge(dve, p - 1)  # pv[u] free
                if s == 0:
                    if hb == 0:
                        tensor.wait_ge(diP, 48)      # V prefix: pairs 0-3
                    elif hb == 4:
                        tensor.wait_ge(diV[0], 32)   # slot-0 V remainder
                elif hb == 0:
                    # slot-0 remainder counts 32 on diV[0]
                    tensor.wait_ge(diV[su], 32 * (1 - su) + 48 * (s // 2 + su))
                mms = pv_mms(p)
                # everything below needs only exp(p) done; scores(p+2) are
                # interleaved so pe_s fires mid-iteration, keeping ACT fed
                tensor.wait_ge(act, p + 1)
                emit_pv_range(p, mms, 0, 4)   # A: sp1 sp2 G loc1
                if p + 2 < NP:
                    emit_scores(p + 2, 0, 6)  # sp x4, G, LOC1
                emit_pv_range(p, mms, 4, 12)  # A: loc0 loc2; B: all
                if p + 2 < NP:
                    emit_scores(p + 2, 6, 9)  # LOC0 LOC2 LOC3

        @block.scalar
        def _(scalar):
            # one exp per pair: ACT is the pacer and each ACTIVATE pays a
            # 143ns PSUM-access bubble, so a single op is cheapest.  The PE
            # side stays busy because scores(p+2) interleave into PV(p).
            # no pe_v wait needed: pe_s >= 2p+2 means scores(p) mm8 is done,
            # which the in-order PE completed after PV(p-2)'s last matmul,
            # so pp[u] is already free.
            for p in range(NP):
                u = p % 2
                nc.scalar.activation(
                    pp[u][:, 0:1536], psS[u][:, 0:1536], EXP, scale=0.125
                ).wait_op(pe_s, 2 * p + 2, GE).then_inc(act, 1)

        @block.vector
        def _(vector):
            # zero rows 64:128 of the K-side tensors once, overlapped with
            # the slot-0 DMA (DVE is idle until the first normalize anyway)
            for tt in (qt_t[0], qt_t[1], lkt_t[0], lkt_t[1],
                       skt_t[0], skt_t[1], gkt_t[0], gkt_t[1]):
                nc.vector.memzero(tt[64:128, :])
            nc.vector.drain()
            nc.vector.nop().then_inc(iz, 1)
            for p in range(NP):
                u = p % 2
                w = p % OBN
                if p >= OBN:
                    vector.wait_ge(st, 16 * (p - OBN + 1))  # ob[w] stored
                nc.vector.reciprocal(rec[u][:, 0:1], pv[u][:, 64:65]).wait_op(
                    pe_v, p + 1, GE
                )
                nc.vector.reciprocal(rec[u][:, 1:2], pv[u][:, 192:193])
                nc.vector.drain()  # DVE pipeline RAW: rec written, read next
                nc.vector.tensor_mul(
                    ob[w][:, 0:64], pv[u][:, 0:64],
                    rec[u][:, 0:1].broadcast_to([128, 64]),
                )
                nc.vector.tensor_mul(
                    ob[w][:, 64:128], pv[u][:, 128:192],
                    rec[u][:, 1:2].broadcast_to([128, 64]),
                ).then_inc(dve, 1)

        @block.sync
        def _(sync):
            for p in range(NP):
                s, hb = divmod(p, PPS)
                b = 2 * hb
                dst = o[s, b * B : (b + 2) * B, :].rearrange(
                    "(blk q) d -> q blk d", blk=2
                )
                src = ob[p % OBN][:, 0:128].rearrange("q (blk d) -> q blk d", blk=2)
                sync.dma_start(dst, src).wait_op(dve, p + 1, GE).then_inc(st, 16)
            sync.wait_ge(st, 16 * NP)

    return nc


def _prepare(inputs):
    import ml_dtypes

    bf = ml_dtypes.bfloat16
    f = np.float32
    q = np.asarray(inputs["query_layer"], f).reshape(NH, T, D)
    k = np.asarray(inputs["key_layer"], f).reshape(NH, T, D)
    v = np.asarray(inputs["value_layer"], f).reshape(NH, T, D)
    sk = np.asarray(inputs["sparse_key"], f).reshape(NH, TSP, D)
    svv = np.asarray(inputs["sparse_value"], f).reshape(NH, TSP, D)
    gk = np.asarray(inputs["global_key"], f).reshape(NH, G, D)
    gvv = np.asarray(inputs["global_value"], f).reshape(NH, G, D)
    am = np.repeat(np.asarray(inputs["attention_mask"], f)[:, 0, 0, :], H, 0)
    sm = np.repeat(np.asarray(inputs["sparse_mask"], f)[:, 0, 0, :], H, 0)
    gm = np.repeat(np.asarray(inputs["global_mask"], f)[:, 0, 0, :], H, 0)

    qt = np.ascontiguousarray(q.transpose(0, 2, 1)).astype(bf)

    lkt = np.zeros((NH, 64, LKT_W), f)
    lkt[:, :, B : B + T] = k.transpose(0, 2, 1)
    lkt = lkt.astype(bf)

    skt = np.zeros((NH, 64, SKT_W), f)
    skt[:, :, 160 : 160 + TSP] = sk.transpose(0, 2, 1)
    skt = skt.astype(bf)

    gkt = np.zeros((NH, 64, 128), f)
    gkt[:, :, :G] = gk.transpose(0, 2, 1)
    gkt = gkt.astype(bf)

    # V_aug rows scaled by exp(mask); pad rows are all-zero
    em_l = np.zeros((NH, LKT_W), f)
    em_l[:, B : B + T] = np.exp(am)
    lvp = np.zeros((NH, LKT_W, 65), f)
    lvp[:, B : B + T, :64] = v
    lvp[:, :, 64] = 1.0
    lvp *= em_l[:, :, None]
    lvp = np.ascontiguousarray(
        lvp.reshape(NH, LV_C, 128, 65).transpose(0, 2, 1, 3)
    ).reshape(NH, 128, LV_C * 65).astype(bf)

    SVP_W = 96 + SV_C * 128
    em_s = np.zeros((NH, SVP_W), f)
    em_s[:, 160 : 160 + TSP] = np.exp(sm)
    sv_pad = np.zeros((NH, SVP_W, 65), f)
    sv_pad[:, 160 : 160 + TSP, :64] = svv
    sv_pad[:, :, 64] = 1.0
    sv_pad *= em_s[:, :, None]
    svp = np.empty((NH, 4, 128, SV_C, 65), f)
    for p in range(4):
        svp[:, p] = (
            sv_pad[:, 32 * p : 32 * p + SV_C * 128]
            .reshape(NH, SV_C, 128, 65)
            .transpose(0, 2, 1, 3)
        )
    # chunk-major columns [chunk, phase, 65]: the first-chunks prefix is one
    # contiguous slice, so slot 0 can gate pairs 0-3 on a single ~500KB load
    svp = np.ascontiguousarray(svp.transpose(0, 2, 3, 1, 4)).reshape(
        NH, 128, 4 * SV_C * 65
    ).astype(bf)

    gvp = np.zeros((NH, 128, 65), f)
    gvp[:, :G, :64] = gvv
    gvp[:, :G, 64] = 1.0
    gvp[:, :G] *= np.exp(gm)[:, :, None]
    gvp = gvp.astype(bf)

    return [
        {
            "qt": qt[c * SL : (c + 1) * SL],
            "lkt": lkt[c * SL : (c + 1) * SL],
            "skt": skt[c * SL : (c + 1) * SL],
            "gkt": gkt[c * SL : (c + 1) * SL],
            "lv": lvp[c * SL : (c + 1) * SL],
            "sv": svp[c * SL : (c + 1) * SL],
            "gv": gvp[c * SL : (c + 1) * SL],
        }
        for c in range(NCORES)
    ]


_NC_CACHE = {}
LAST_RESULTS = None


def kernel(**inputs):
    global LAST_RESULTS
    if "nc" not in _NC_CACHE:
        _NC_CACHE["nc"] = _build_bass()
    nc = _NC_CACHE["nc"]
    in_maps = _prepare(inputs)
    res = run_bass_kernel_spmd(nc, in_maps, core_ids=list(range(NCORES)))
    LAST_RESULTS = res
    out = np.empty((NH, T, D), np.float32)
    for c in range(NCORES):
        out[c * SL : (c + 1) * SL] = res.results[c]["o"]
    return out.reshape(N, H, T, D)
